# revision 22
# baseline (speedup 1.0000x reference)
"""nn_CNNxCNN_Attention Trainium2 Bass kernel (self-contained).

Row-sharded data parallelism over 8 cores (core i: batch i//4, rows
128*(i%4)..+128); the only cross-core exchange is an AllReduce of
per-head Gram matrices + squared norms (L2 normalization folds into
the Gram).

Fused 1x1+3x3dw conv via K-packed bf16 matmuls: x is staged in SBUF
2.67x (three column-shifted copies on partitions 0-47/48-95/96-127 plus
a small row-shifted remainder tile) so each matmul contracts K=128 over
(channel x tap) -- 4 matmuls per 512-wide output row per output half
(qk 96 / v 48) instead of 18 K=48 matmuls.

Conv outputs are unsorted into compact persistent staging (qkb_all /
vstg_all) with (nh, nw, h1, w1) free layout; per-head expansion gathers
run on the gpsimd SWDGE queue; q/k chunk transposes use the DMA
crossbar transpose (dma_start_transpose) split across sync/scalar
issue queues; Gram + squared norms stream per superblock overlapping
the next superblock's conv. Softmax folds the L2 norms, temperature,
and 1/rowsum into the transposed attention matrix; attn@v psum is
regrouped by DMA into a (head,chan)-partition tile so the 1x1 proj
runs with contiguous rhs; y is written HBM-blocked and un-blocked on
the host.
"""

from dataclasses import dataclass

import numpy as np
import ml_dtypes

import concourse.bass as bass
import concourse.bacc as bacc
import concourse.mybir as mybir
from concourse.tile import TileContext

F32 = mybir.dt.float32
BF16 = mybir.dt.bfloat16
AF = mybir.ActivationFunctionType
ALU = mybir.AluOpType

HEADS = 8
CIN = 48
W = 512
W1 = 128          # coarse token columns
HLOC = 128        # rows per core
NSB = 4           # superblocks of 32 rows
NCK = 8           # attn/proj token chunks (4 h1-rows of 128 w1 each)


@dataclass
class Cfg:
    W: int = 512
    HLOC: int = 128
    n_cores: int = 8


def build_nc(temps, n_cores=8, stage=3, parts=15):
    # stage: 1=conv+unsort only, 2=+gram (zeros out), 3=full
    # parts bitmask: 1=gathers 2=transposes 4=gram-mm 8=squares
    nc = bacc.Bacc("TRN2", target_bir_lowering=False, debug=False,
                   num_devices=n_cores)

    x_in = nc.declare_dram_parameter("x", [CIN, HLOC + 2, W + 2], BF16,
                                     isOutput=False)
    w3a_in = nc.declare_dram_parameter("w3a", [128, 3, 96], BF16,
                                       isOutput=False)
    w3av_in = nc.declare_dram_parameter("w3av", [128, 3, 48], BF16,
                                        isOutput=False)
    w3b_in = nc.declare_dram_parameter("w3b", [48, 96], BF16, isOutput=False)
    w3bv_in = nc.declare_dram_parameter("w3bv", [48, 48], BF16,
                                        isOutput=False)
    wp_in = nc.declare_dram_parameter("wp", [48, 48], BF16, isOutput=False)
    id_in = nc.declare_dram_parameter("idm", [128, 128], BF16, isOutput=False)
    # blocked output: [ch, nh, nw, h1loc, w1]; host re-interleaves
    y_out = nc.declare_dram_parameter("y", [CIN, 4, 4, 32, 128], F32,
                                      isOutput=True)

    cc_in = nc.dram_tensor("cc_in", [96, 784], F32)
    cc_out = nc.dram_tensor("cc_out", [96, 784], F32)
    half = n_cores // 2
    groups = [list(range(0, half)), list(range(half, n_cores))]

    with TileContext(nc) as tc:
        with tc.tile_pool(name="persist", bufs=1) as pp:
            # ---- constants ----
            w3a = pp.tile([128, 3, 96], BF16)
            w3av = pp.tile([128, 3, 48], BF16)
            w3b = pp.tile([48, 96], BF16)
            w3bv = pp.tile([48, 48], BF16)
            wp = pp.tile([48, 48], BF16)
            idm = pp.tile([128, 128], BF16)
            nc.sync.dma_start(out=w3a[:], in_=w3a_in[:])
            nc.sync.dma_start(out=w3av[:], in_=w3av_in[:])
            nc.sync.dma_start(out=w3b[:], in_=w3b_in[:])
            nc.sync.dma_start(out=w3bv[:], in_=w3bv_in[:])
            nc.sync.dma_start(out=wp[:], in_=wp_in[:])
            nc.sync.dma_start(out=idm[:], in_=id_in[:])

            # ---- persistent v staging: halves on partitions 0-47/64-111
            # free layout (nh, nw, h1-in-half, w1) ----
            v_sorted = pp.tile([112, 4, 4, 16, W1], BF16)
            sq_part = pp.tile([96, 16, NSB], F32)   # (q8,k8) x superblock
            gsq = pp.tile([96, 784], F32)           # G(768) + sq(16)

            # ================= conv + gram streaming =================
            with tc.tile_pool(name="conv", bufs=2) as cp, \
                 tc.tile_pool(name="psum_conv", bufs=1, space="PSUM") as pcp:
                xrs, xr2s = {}, {}

                def load_group(g):
                    # xr: parts 0-47 = x (dx tap 0), 48-95 = x shifted one
                    # col (dx tap 1), 96-127 = x[0:32] shifted two (dx tap 2)
                    xr = cp.tile([128, 18, 512], BF16, tag="xr")
                    r0 = 16 * g
                    nc.sync.dma_start(out=xr[0:48, :, :],
                                      in_=x_in[:, r0:r0 + 18, 0:512])
                    nc.sync.dma_start(out=xr[48:96, :, :],
                                      in_=x_in[:, r0:r0 + 18, 1:513])
                    nc.sync.dma_start(out=xr[96:128, :, :],
                                      in_=x_in[0:32, r0:r0 + 18, 2:514])
                    # xr2: remainder taps (dy,2) for channels 32-47,
                    # partition block dy holds rows r0+dy+hh*8 .. +8
                    halves = []
                    for hh in range(2):
                        x2 = cp.tile([48, 8, 512], BF16, tag="xr2")
                        for dy in range(3):
                            rb = r0 + dy + 8 * hh
                            nc.scalar.dma_start(
                                out=x2[16 * dy:16 * dy + 16, :, :],
                                in_=x_in[32:48, rb:rb + 8, 2:514])
                        halves.append(x2)
                    xrs[g] = xr
                    xr2s[g] = halves

                load_group(0)
                for sb in range(NSB):
                    # superblock qk staging, (nh, nw, gg, grprow, w1)
                    qkb = cp.tile([96, 4, 4, 2, 4, W1], BF16, tag="qkb")
                    for gg in range(2):
                        g = 2 * sb + gg
                        xr, xr2 = xrs.pop(g), xr2s.pop(g)
                        for yy in range(16):
                            if yy == 8 and g + 1 < 2 * NSB:
                                load_group(g + 1)
                            x2 = xr2[yy // 8]
                            yl = yy % 8
                            y = 16 * g + yy
                            vb = 0 if y < 64 else 64
                            ps_qk = pcp.tile([96, W], F32, tag="ps_qk",
                                             bufs=2)
                            ps_v = pcp.tile([112, W], F32, tag="ps_v",
                                            bufs=2)
                            for dy in range(3):
                                rhs = xr[0:128, yy + dy, 0:512]
                                nc.tensor.matmul(ps_qk[:], w3a[:, dy, :], rhs,
                                                 start=(dy == 0), stop=False)
                            nc.tensor.matmul(ps_qk[:], w3b[:],
                                             x2[0:48, yl, :],
                                             start=False, stop=True)
                            vs = ps_v[vb:vb + 48, :]
                            for dy in range(3):
                                rhs = xr[0:128, yy + dy, 0:512]
                                nc.tensor.matmul(vs, w3av[:, dy, :], rhs,
                                                 start=(dy == 0), stop=False,
                                                 tile_position=(0, vb))
                            nc.tensor.matmul(vs, w3bv[:],
                                             x2[0:48, yl, :],
                                             start=False, stop=True,
                                             tile_position=(0, vb))
                            nh, grprow = yy % 4, yy // 4
                            src_qk = ps_qk.rearrange("p (w1 nw) -> p nw w1",
                                                     nw=4)
                            nc.scalar.copy(qkb[:, nh, :, gg, grprow, :],
                                           src_qk)
                            src_v = vs.rearrange("p (w1 nw) -> p nw w1",
                                                 nw=4)
                            nc.vector.tensor_copy(
                                v_sorted[vb:vb + 48, nh, :,
                                         (y % 64) // 4, :], src_v)

                    # ---- per-superblock gram (overlaps next sb conv) ----
                    for h in range(HEADS if stage >= 2 else 0):
                        qcb = cp.tile([96, 8, W1], BF16, tag="qcb")
                        kcb = cp.tile([96, 8, W1], BF16, tag="kcb")
                        if parts & 1:
                            nc.scalar.dma_start(
                                out=qcb[:], in_=qkb[6 * h:6 * h + 6])
                            nc.scalar.dma_start(
                                out=kcb[:],
                                in_=qkb[48 + 6 * h:48 + 6 * h + 6])
                        else:
                            nc.vector.memset(qcb[:], 0.125)
                            nc.vector.memset(kcb[:], 0.125)
                        qT = cp.tile([128, 8, 96], BF16, tag="qT")
                        kT = cp.tile([128, 8, 96], BF16, tag="kT")
                        if parts & 2:
                            nc.sync.dma_start_transpose(
                                qT[:], qcb.rearrange("p a b -> p (a b)"))
                            nc.scalar.dma_start_transpose(
                                kT[:], kcb.rearrange("p a b -> p (a b)"))
                        else:
                            nc.vector.memset(qT[:], 0.125)
                            nc.vector.memset(kT[:], 0.125)
                        gps = pcp.tile([96, 96], F32, tag="gps", bufs=2)
                        if parts & 4:
                            for ck in range(8):
                                nc.tensor.matmul(gps[:], kT[:, ck, :],
                                                 qT[:, ck, :],
                                                 start=(ck == 0),
                                                 stop=(ck == 7))
                        else:
                            nc.tensor.matmul(gps[:], kT[:, 0, :], qT[:, 0, :],
                                             start=True, stop=True)
                        if sb == 0:
                            nc.vector.tensor_copy(gsq[:, 96 * h:96 * h + 96],
                                                  gps[:])
                        else:
                            nc.vector.tensor_tensor(
                                gsq[:, 96 * h:96 * h + 96],
                                gsq[:, 96 * h:96 * h + 96], gps[:],
                                op=ALU.add)
                        if parts & 8:
                            sc0 = cp.tile([96, 8 * W1], BF16, tag="sc0",
                                          bufs=2)
                            nc.gpsimd.tensor_tensor(
                                sc0[:], qcb.rearrange("p a b -> p (a b)"),
                                qcb.rearrange("p a b -> p (a b)"),
                                op=ALU.mult)
                            nc.vector.tensor_reduce(
                                sq_part[:, h, sb:sb + 1], sc0[:],
                                axis=mybir.AxisListType.X, op=ALU.add)
                            sc1 = cp.tile([96, 8 * W1], BF16, tag="sc1",
                                          bufs=2)
                            nc.gpsimd.tensor_tensor(
                                sc1[:], kcb.rearrange("p a b -> p (a b)"),
                                kcb.rearrange("p a b -> p (a b)"),
                                op=ALU.mult)
                            nc.vector.tensor_reduce(
                                sq_part[:, 8 + h, sb:sb + 1], sc1[:],
                                axis=mybir.AxisListType.X, op=ALU.add)
                        else:
                            nc.vector.memset(sq_part[:, h, sb:sb + 1], 1.0)
                            nc.vector.memset(sq_part[:, 8 + h, sb:sb + 1],
                                             1.0)

            # ================= collective + tail =================
            if stage < 3:
                with tc.tile_pool(name="zf", bufs=1) as zp:
                    zt = zp.tile([CIN, 4, 4, W], F32)
                    nc.vector.memset(zt[:], 0.0)
                    for ck in range(NCK):
                        nc.sync.dma_start(
                            out=y_out[:, :, :, 4 * ck:4 * ck + 4, :],
                            in_=zt[:])
            if stage >= 3:
              with tc.tile_pool(name="tailp", bufs=1) as tp:
                nc.vector.tensor_reduce(gsq[:, 768:784], sq_part[:],
                                        axis=mybir.AxisListType.X, op=ALU.add)
                nc.sync.dma_start(out=cc_in[:], in_=gsq[:])
                nc.gpsimd.collective_compute(
                    "AllReduce", ALU.add, replica_groups=groups,
                    ins=[cc_in[:]], outs=[cc_out[:]])
                gsq_r = tp.tile([96, 784], F32)
                nc.sync.dma_start(out=gsq_r[:], in_=cc_out[:])

                # vcb expansion gathers (hide under the collective)
                vcbs = {}
                for h in range(HEADS):
                    vcbs[h] = tp.tile([96, 32, W1], BF16, name=f"vcb{h}")
                    nc.sync.dma_start(out=vcbs[h][:, 0:16, :],
                                      in_=v_sorted[6 * h:6 * h + 6])
                    nc.sync.dma_start(
                        out=vcbs[h][:, 16:32, :],
                        in_=v_sorted[64 + 6 * h:64 + 6 * h + 6])

                # ============ softmax (norm + 1/rowsum folded) ============
                nrm = tp.tile([96, 16], F32)
                rs = tp.tile([96, 16], F32)
                nc.scalar.sqrt(nrm[:], gsq_r[:, 768:784])
                nc.vector.tensor_scalar_max(nrm[:], nrm[:], 1e-12)
                nc.vector.reciprocal(rs[:], nrm[:])

                atTs = {}
                with tc.tile_pool(name="smx", bufs=2) as sp, \
                     tc.tile_pool(name="psum_smx", bufs=2,
                                  space="PSUM") as psp:
                    for h in range(HEADS):
                        # gsq holds G^T[e,d]; scale rows (e) by k-norms
                        hbf = sp.tile([96, 96], BF16, tag="hbf")
                        nc.vector.tensor_scalar_mul(
                            hbf[:], gsq_r[:, 96 * h:96 * h + 96],
                            rs[:, 8 + h:8 + h + 1])
                        ht_ps = psp.tile([96, 96], BF16, tag="ht_ps")
                        nc.tensor.transpose(ht_ps[:], hbf[:], idm[0:96, 0:96])
                        sd = sp.tile([96, 1], F32, tag="sd")
                        nc.vector.tensor_scalar_mul(sd[:], rs[:, h:h + 1],
                                                    float(temps[h]))
                        aexp = sp.tile([96, 96], BF16, tag="aexp")
                        rowsum = sp.tile([96, 1], F32, tag="rowsum")
                        nc.scalar.activation(aexp[:], ht_ps[:], AF.Exp,
                                             scale=sd[:], accum_out=rowsum[:])
                        rinv = sp.tile([96, 1], F32, tag="rinv")
                        nc.vector.reciprocal(rinv[:], rowsum[:])
                        aexp2 = sp.tile([96, 96], BF16, tag="aexp2")
                        nc.vector.tensor_scalar_mul(aexp2[:], aexp[:],
                                                    rinv[:])
                        at_ps = psp.tile([96, 96], BF16, tag="at_ps")
                        nc.tensor.transpose(at_ps[:], aexp2[:],
                                            idm[0:96, 0:96])
                        atT = tp.tile([96, 96], BF16, name=f"atT{h}")
                        nc.scalar.copy(atT[:], at_ps[:])
                        atTs[h] = atT

                # ============ attn@v + regroup + proj ============
                with tc.tile_pool(name="attn", bufs=2) as ap, \
                     tc.tile_pool(name="psum_attn", bufs=2,
                                  space="PSUM") as pap:
                    for ck in range(NCK):
                        z = ap.tile([48, 4, 4, W], BF16, tag="z")
                        for h in range(HEADS):
                            av_ps = pap.tile([96, W], F32, tag="av_ps",
                                             bufs=2)
                            rhs = vcbs[h][:, 4 * ck:4 * ck + 4, :].rearrange(
                                "p a b -> p (a b)")
                            nc.tensor.matmul(av_ps[:], atTs[h][:], rhs,
                                             start=True, stop=True)
                            och = ap.tile([96, W], BF16, tag="och", bufs=4)
                            if h % 2 == 0:
                                nc.scalar.copy(och[:], av_ps[:])
                            else:
                                nc.vector.tensor_copy(och[:], av_ps[:])
                            if h % 2 == 0:
                                nc.scalar.dma_start(
                                    out=z[6 * h:6 * h + 6], in_=och[:])
                            else:
                                nc.sync.dma_start(
                                    out=z[6 * h:6 * h + 6], in_=och[:])
                        for nh in range(4):
                            ystg = ap.tile([48, 4, W], F32, tag="ystg",
                                           bufs=4)
                            for nw in range(4):
                                pj = pap.tile([48, W], F32, tag="pj", bufs=2)
                                nc.tensor.matmul(pj[:], wp[:],
                                                 z[0:48, nh, nw, :],
                                                 start=True, stop=True)
                                if nw % 2 == 0:
                                    nc.scalar.copy(ystg[:, nw, :], pj[:])
                                else:
                                    nc.vector.tensor_copy(ystg[:, nw, :],
                                                          pj[:])
                            nc.sync.dma_start(
                                out=y_out[:, nh, :, 4 * ck:4 * ck + 4, :],
                                in_=ystg[:])

    nc.compile()
    nc.generate_event_semaphores()
    return nc


# ====================== host-side helpers ======================

def host_prepare(x, qkv_w, dw_w, proj_w, cfg: Cfg):
    """Full inputs -> list of per-core input dicts."""
    bf = ml_dtypes.bfloat16
    b, cin, H, Wf = x.shape
    assert cin == CIN and Wf == cfg.W
    n = cfg.n_cores
    per_b = n // b
    assert cfg.HLOC * per_b == H

    w1 = qkv_w[:, :, 0, 0].astype(np.float32)          # [144, 48]
    dw = dw_w[:, 0, :, :].astype(np.float32)           # [144, 3, 3]
    # w3T[ic, dy, dx, oc] = w1[oc, ic] * dw[oc, dy, dx]
    w3T = np.einsum('oi,oyx->iyxo', w1, dw)            # [48, 3, 3, 144]

    w3a = np.zeros((128, 3, 96), np.float32)
    w3av = np.zeros((128, 3, 48), np.float32)
    for dy in range(3):
        w3a[0:48, dy] = w3T[:, dy, 0, 0:96]
        w3a[48:96, dy] = w3T[:, dy, 1, 0:96]
        w3a[96:128, dy] = w3T[0:32, dy, 2, 0:96]
        w3av[0:48, dy] = w3T[:, dy, 0, 96:144]
        w3av[48:96, dy] = w3T[:, dy, 1, 96:144]
        w3av[96:128, dy] = w3T[0:32, dy, 2, 96:144]
    w3b = np.zeros((48, 96), np.float32)
    w3bv = np.zeros((48, 48), np.float32)
    for dy in range(3):
        w3b[16 * dy:16 * dy + 16] = w3T[32:48, dy, 2, 0:96]
        w3bv[16 * dy:16 * dy + 16] = w3T[32:48, dy, 2, 96:144]

    wp = np.ascontiguousarray(proj_w[:, :, 0, 0].astype(np.float32).T)
    idm = np.eye(128, dtype=np.float32)

    xp = np.zeros((b, cin, H + 2, cfg.W + 2), dtype=np.float32)
    xp[:, :, 1:H + 1, 1:cfg.W + 1] = x
    xp = xp.astype(bf)

    const = {"w3a": w3a.astype(bf), "w3av": w3av.astype(bf),
             "w3b": w3b.astype(bf), "w3bv": w3bv.astype(bf),
             "wp": wp.astype(bf), "idm": idm.astype(bf)}
    in_maps = []
    for i in range(n):
        bi, r = i // per_b, i % per_b
        xs = np.ascontiguousarray(
            xp[bi, :, cfg.HLOC * r:cfg.HLOC * r + cfg.HLOC + 2, :])
        m = {"x": xs}
        m.update(const)
        in_maps.append(m)
    return in_maps


def host_assemble(results, cfg: Cfg, b, H):
    n = cfg.n_cores
    per_b = n // b
    y = np.empty((b, CIN, H, cfg.W), dtype=np.float32)
    for i in range(n):
        bi, r = i // per_b, i % per_b
        yb = results[i]["y"]                     # [48, 4, 4, 32, 128]
        yc = yb.transpose(0, 3, 1, 4, 2).reshape(CIN, cfg.HLOC, cfg.W)
        y[bi, :, cfg.HLOC * r:cfg.HLOC * (r + 1), :] = yc
    return y


_NC_CACHE = {}


def kernel(x, qkv_w, dw_w, proj_w, temperature):
    x = np.asarray(x, dtype=np.float32)
    qkv_w = np.asarray(qkv_w, dtype=np.float32)
    dw_w = np.asarray(dw_w, dtype=np.float32)
    proj_w = np.asarray(proj_w, dtype=np.float32)
    temperature = np.asarray(temperature, dtype=np.float32)

    from concourse.bass_utils import run_bass_kernel_spmd

    cfg = Cfg(W=512, HLOC=128, n_cores=8)
    temps = tuple(float(t) for t in temperature[:, 0, 0])
    if temps not in _NC_CACHE:
        _NC_CACHE[temps] = build_nc(list(temps), cfg.n_cores)
    nc = _NC_CACHE[temps]
    in_maps = host_prepare(x, qkv_w, dw_w, proj_w, cfg)
    res = run_bass_kernel_spmd(nc, in_maps, list(range(cfg.n_cores)))
    return host_assemble(res.results, cfg, x.shape[0], x.shape[2])


# revision 23
# speedup vs baseline: 1.0576x; 1.0576x over previous
"""nn_CNNxCNN_Attention Trainium2 Bass kernel (self-contained).

Row-sharded data parallelism over 8 cores (core i: batch i//4, rows
128*(i%4)..+128); the only cross-core exchange is an AllReduce of
per-head Gram matrices + squared norms (L2 normalization folds into
the Gram).

Fused 1x1+3x3dw conv via K-packed bf16 matmuls: x is staged in SBUF
2.67x (three column-shifted copies on partitions 0-47/48-95/96-127 plus
a small row-shifted remainder tile) so each matmul contracts K=128 over
(channel x tap) -- 4 matmuls per 512-wide output row per output half
(qk 96 / v 48) instead of 18 K=48 matmuls.

Conv outputs are unsorted into compact persistent staging (qkb_all /
vstg_all) with (nh, nw, h1, w1) free layout; per-head expansion gathers
run on the gpsimd SWDGE queue; q/k chunk transposes use the DMA
crossbar transpose (dma_start_transpose) split across sync/scalar
issue queues; Gram + squared norms stream per superblock overlapping
the next superblock's conv. Softmax folds the L2 norms, temperature,
and 1/rowsum into the transposed attention matrix; attn@v psum is
regrouped by DMA into a (head,chan)-partition tile so the 1x1 proj
runs with contiguous rhs; y is written HBM-blocked and un-blocked on
the host.
"""

from dataclasses import dataclass

import numpy as np
import ml_dtypes

import concourse.bass as bass
import concourse.bacc as bacc
import concourse.mybir as mybir
from concourse.tile import TileContext

F32 = mybir.dt.float32
BF16 = mybir.dt.bfloat16
AF = mybir.ActivationFunctionType
ALU = mybir.AluOpType

HEADS = 8
CIN = 48
W = 512
W1 = 128          # coarse token columns
HLOC = 128        # rows per core
NSB = 4           # superblocks of 32 rows
NCK = 8           # attn/proj token chunks (4 h1-rows of 128 w1 each)


@dataclass
class Cfg:
    W: int = 512
    HLOC: int = 128
    n_cores: int = 8


def build_nc(temps, n_cores=8, stage=3, parts=15):
    # stage: 1=conv+unsort only, 2=+gram (zeros out), 3=full
    # parts bitmask: 1=gathers 2=transposes 4=gram-mm 8=squares
    nc = bacc.Bacc("TRN2", target_bir_lowering=False, debug=False,
                   num_devices=n_cores)

    x_in = nc.declare_dram_parameter("x", [CIN, HLOC + 2, W + 2], BF16,
                                     isOutput=False)
    w3a_in = nc.declare_dram_parameter("w3a", [128, 3, 96], BF16,
                                       isOutput=False)
    w3av_in = nc.declare_dram_parameter("w3av", [128, 3, 48], BF16,
                                        isOutput=False)
    w3b_in = nc.declare_dram_parameter("w3b", [48, 96], BF16, isOutput=False)
    w3bv_in = nc.declare_dram_parameter("w3bv", [48, 48], BF16,
                                        isOutput=False)
    wp_in = nc.declare_dram_parameter("wp", [48, 48], BF16, isOutput=False)
    id_in = nc.declare_dram_parameter("idm", [128, 128], BF16, isOutput=False)
    # blocked output: [ch, nh, nw, h1loc, w1]; host re-interleaves
    y_out = nc.declare_dram_parameter("y", [CIN, 4, 4, 32, 128], BF16,
                                      isOutput=True)

    cc_in = nc.dram_tensor("cc_in", [96, 784], F32)
    cc_out = nc.dram_tensor("cc_out", [96, 784], F32)
    half = n_cores // 2
    groups = [list(range(0, half)), list(range(half, n_cores))]

    with TileContext(nc) as tc:
        with tc.tile_pool(name="persist", bufs=1) as pp:
            # ---- constants ----
            w3a = pp.tile([128, 3, 96], BF16)
            w3av = pp.tile([128, 3, 48], BF16)
            w3b = pp.tile([48, 96], BF16)
            w3bv = pp.tile([48, 48], BF16)
            wp = pp.tile([48, 48], BF16)
            idm = pp.tile([128, 128], BF16)
            nc.sync.dma_start(out=w3a[:], in_=w3a_in[:])
            nc.sync.dma_start(out=w3av[:], in_=w3av_in[:])
            nc.sync.dma_start(out=w3b[:], in_=w3b_in[:])
            nc.sync.dma_start(out=w3bv[:], in_=w3bv_in[:])
            nc.sync.dma_start(out=wp[:], in_=wp_in[:])
            nc.sync.dma_start(out=idm[:], in_=id_in[:])

            # ---- persistent v staging: halves on partitions 0-47/64-111
            # free layout (nh, nw, h1-in-half, w1) ----
            v_sorted = pp.tile([112, 4, 4, 16, W1], BF16)
            sq_part = pp.tile([96, 16, NSB], F32)   # (q8,k8) x superblock
            gsq = pp.tile([96, 784], F32)           # G(768) + sq(16)

            # ================= conv + gram streaming =================
            with tc.tile_pool(name="conv", bufs=2) as cp, \
                 tc.tile_pool(name="psum_conv", bufs=1, space="PSUM") as pcp:
                xrs, xr2s = {}, {}

                def load_group(g):
                    # xr: parts 0-47 = x (dx tap 0), 48-95 = x shifted one
                    # col (dx tap 1), 96-127 = x[0:32] shifted two (dx tap 2)
                    xr = cp.tile([128, 18, 512], BF16, tag="xr")
                    r0 = 16 * g
                    nc.sync.dma_start(out=xr[0:48, :, :],
                                      in_=x_in[:, r0:r0 + 18, 0:512])
                    nc.sync.dma_start(out=xr[48:96, :, :],
                                      in_=x_in[:, r0:r0 + 18, 1:513])
                    nc.sync.dma_start(out=xr[96:128, :, :],
                                      in_=x_in[0:32, r0:r0 + 18, 2:514])
                    # xr2: remainder taps (dy,2) for channels 32-47,
                    # partition block dy holds rows r0+dy+hh*8 .. +8
                    halves = []
                    for hh in range(2):
                        x2 = cp.tile([48, 8, 512], BF16, tag="xr2")
                        for dy in range(3):
                            rb = r0 + dy + 8 * hh
                            nc.scalar.dma_start(
                                out=x2[16 * dy:16 * dy + 16, :, :],
                                in_=x_in[32:48, rb:rb + 8, 2:514])
                        halves.append(x2)
                    xrs[g] = xr
                    xr2s[g] = halves

                load_group(0)
                for sb in range(NSB):
                    # superblock qk staging, (nh, nw, gg, grprow, w1)
                    qkb = cp.tile([96, 4, 4, 2, 4, W1], BF16, tag="qkb")
                    for gg in range(2):
                        g = 2 * sb + gg
                        xr, xr2 = xrs.pop(g), xr2s.pop(g)
                        for yy in range(16):
                            if yy == 8 and g + 1 < 2 * NSB:
                                load_group(g + 1)
                            x2 = xr2[yy // 8]
                            yl = yy % 8
                            y = 16 * g + yy
                            vb = 0 if y < 64 else 64
                            ps_qk = pcp.tile([96, W], F32, tag="ps_qk",
                                             bufs=3)
                            ps_v = pcp.tile([112, W], F32, tag="ps_v",
                                            bufs=3)
                            for dy in range(3):
                                rhs = xr[0:128, yy + dy, 0:512]
                                nc.tensor.matmul(ps_qk[:], w3a[:, dy, :], rhs,
                                                 start=(dy == 0), stop=False)
                            nc.tensor.matmul(ps_qk[:], w3b[:],
                                             x2[0:48, yl, :],
                                             start=False, stop=True)
                            vs = ps_v[vb:vb + 48, :]
                            for dy in range(3):
                                rhs = xr[0:128, yy + dy, 0:512]
                                nc.tensor.matmul(vs, w3av[:, dy, :], rhs,
                                                 start=(dy == 0), stop=False,
                                                 tile_position=(0, vb))
                            nc.tensor.matmul(vs, w3bv[:],
                                             x2[0:48, yl, :],
                                             start=False, stop=True,
                                             tile_position=(0, vb))
                            nh, grprow = yy % 4, yy // 4
                            src_qk = ps_qk.rearrange("p (w1 nw) -> p nw w1",
                                                     nw=4)
                            nc.vector.tensor_copy(
                                qkb[:, nh, :, gg, grprow, :], src_qk)
                            src_v = vs.rearrange("p (w1 nw) -> p nw w1",
                                                 nw=4)
                            nc.vector.tensor_copy(
                                v_sorted[vb:vb + 48, nh, :,
                                         (y % 64) // 4, :], src_v)

                    # ---- per-superblock gram (overlaps next sb conv) ----
                    for h in range(HEADS if stage >= 2 else 0):
                        qcb = cp.tile([96, 8, W1], BF16, tag="qcb")
                        kcb = cp.tile([96, 8, W1], BF16, tag="kcb")
                        if parts & 1:
                            nc.scalar.dma_start(
                                out=qcb[:], in_=qkb[6 * h:6 * h + 6])
                            nc.scalar.dma_start(
                                out=kcb[:],
                                in_=qkb[48 + 6 * h:48 + 6 * h + 6])
                        else:
                            nc.vector.memset(qcb[:], 0.125)
                            nc.vector.memset(kcb[:], 0.125)
                        qT = cp.tile([128, 8, 96], BF16, tag="qT")
                        kT = cp.tile([128, 8, 96], BF16, tag="kT")
                        if parts & 2:
                            nc.scalar.dma_start_transpose(
                                qT[:], qcb.rearrange("p a b -> p (a b)"))
                            nc.scalar.dma_start_transpose(
                                kT[:], kcb.rearrange("p a b -> p (a b)"))
                        else:
                            nc.vector.memset(qT[:], 0.125)
                            nc.vector.memset(kT[:], 0.125)
                        gps = pcp.tile([96, 96], F32, tag="gps", bufs=2)
                        if parts & 4:
                            for ck in range(8):
                                nc.tensor.matmul(gps[:], kT[:, ck, :],
                                                 qT[:, ck, :],
                                                 start=(ck == 0),
                                                 stop=(ck == 7))
                        else:
                            nc.tensor.matmul(gps[:], kT[:, 0, :], qT[:, 0, :],
                                             start=True, stop=True)
                        if sb == 0:
                            nc.vector.tensor_copy(gsq[:, 96 * h:96 * h + 96],
                                                  gps[:])
                        else:
                            nc.vector.tensor_tensor(
                                gsq[:, 96 * h:96 * h + 96],
                                gsq[:, 96 * h:96 * h + 96], gps[:],
                                op=ALU.add)
                        if parts & 8:
                            sc0 = cp.tile([96, 8 * W1], BF16, tag="sc0",
                                          bufs=2)
                            nc.gpsimd.tensor_tensor(
                                sc0[:], qcb.rearrange("p a b -> p (a b)"),
                                qcb.rearrange("p a b -> p (a b)"),
                                op=ALU.mult)
                            nc.vector.tensor_reduce(
                                sq_part[:, h, sb:sb + 1], sc0[:],
                                axis=mybir.AxisListType.X, op=ALU.add)
                            sc1 = cp.tile([96, 8 * W1], BF16, tag="sc1",
                                          bufs=2)
                            nc.scalar.activation(
                                sc1[:], kcb.rearrange("p a b -> p (a b)"),
                                AF.Square,
                                accum_out=sq_part[:, 8 + h, sb:sb + 1])
                        else:
                            nc.vector.memset(sq_part[:, h, sb:sb + 1], 1.0)
                            nc.vector.memset(sq_part[:, 8 + h, sb:sb + 1],
                                             1.0)

            # ================= collective + tail =================
            if stage < 3:
                with tc.tile_pool(name="zf", bufs=1) as zp:
                    zt = zp.tile([CIN, 4, 4, W], BF16)
                    nc.vector.memset(zt[:], 0.0)
                    for ck in range(NCK):
                        nc.sync.dma_start(
                            out=y_out[:, :, :, 4 * ck:4 * ck + 4, :],
                            in_=zt[:])
            if stage >= 3:
              with tc.tile_pool(name="tailp", bufs=1) as tp:
                nc.vector.tensor_reduce(gsq[:, 768:784], sq_part[:],
                                        axis=mybir.AxisListType.X, op=ALU.add)
                nc.sync.dma_start(out=cc_in[:], in_=gsq[:])
                nc.gpsimd.collective_compute(
                    "AllReduce", ALU.add, replica_groups=groups,
                    ins=[cc_in[:]], outs=[cc_out[:]])
                gsq_r = tp.tile([96, 784], F32)
                nc.sync.dma_start(out=gsq_r[:], in_=cc_out[:])

                # vcb expansion gathers (hide under the collective)
                vcbs = {}
                for h in range(HEADS):
                    vcbs[h] = tp.tile([96, 32, W1], BF16, name=f"vcb{h}")
                    nc.sync.dma_start(out=vcbs[h][:, 0:16, :],
                                      in_=v_sorted[6 * h:6 * h + 6])
                    nc.sync.dma_start(
                        out=vcbs[h][:, 16:32, :],
                        in_=v_sorted[64 + 6 * h:64 + 6 * h + 6])

                # ============ softmax (norm + 1/rowsum folded) ============
                nrm = tp.tile([96, 16], F32)
                rs = tp.tile([96, 16], F32)
                nc.scalar.sqrt(nrm[:], gsq_r[:, 768:784])
                nc.vector.tensor_scalar_max(nrm[:], nrm[:], 1e-12)
                nc.vector.reciprocal(rs[:], nrm[:])

                atTs = {}
                with tc.tile_pool(name="smx", bufs=2) as sp, \
                     tc.tile_pool(name="psum_smx", bufs=2,
                                  space="PSUM") as psp:
                    for h in range(HEADS):
                        # gsq holds G^T[e,d]; scale rows (e) by k-norms
                        hbf = sp.tile([96, 96], BF16, tag="hbf")
                        nc.vector.tensor_scalar_mul(
                            hbf[:], gsq_r[:, 96 * h:96 * h + 96],
                            rs[:, 8 + h:8 + h + 1])
                        ht_ps = psp.tile([96, 96], BF16, tag="ht_ps")
                        nc.tensor.transpose(ht_ps[:], hbf[:], idm[0:96, 0:96])
                        sd = sp.tile([96, 1], F32, tag="sd")
                        nc.vector.tensor_scalar_mul(sd[:], rs[:, h:h + 1],
                                                    float(temps[h]))
                        aexp = sp.tile([96, 96], BF16, tag="aexp")
                        rowsum = sp.tile([96, 1], F32, tag="rowsum")
                        nc.scalar.activation(aexp[:], ht_ps[:], AF.Exp,
                                             scale=sd[:], accum_out=rowsum[:])
                        rinv = sp.tile([96, 1], F32, tag="rinv")
                        nc.vector.reciprocal(rinv[:], rowsum[:])
                        aexp2 = sp.tile([96, 96], BF16, tag="aexp2")
                        nc.vector.tensor_scalar_mul(aexp2[:], aexp[:],
                                                    rinv[:])
                        at_ps = psp.tile([96, 96], BF16, tag="at_ps")
                        nc.tensor.transpose(at_ps[:], aexp2[:],
                                            idm[0:96, 0:96])
                        atT = tp.tile([96, 96], BF16, name=f"atT{h}")
                        nc.scalar.copy(atT[:], at_ps[:])
                        atTs[h] = atT

                # ============ attn@v + regroup + proj ============
                with tc.tile_pool(name="attn", bufs=2) as ap, \
                     tc.tile_pool(name="psum_attn", bufs=2,
                                  space="PSUM") as pap:
                    for ck in range(NCK):
                        z = ap.tile([48, 4, 4, W], BF16, tag="z")
                        for h in range(HEADS):
                            av_ps = pap.tile([96, W], F32, tag="av_ps",
                                             bufs=2)
                            rhs = vcbs[h][:, 4 * ck:4 * ck + 4, :].rearrange(
                                "p a b -> p (a b)")
                            nc.tensor.matmul(av_ps[:], atTs[h][:], rhs,
                                             start=True, stop=True)
                            och = ap.tile([96, W], BF16, tag="och", bufs=4)
                            nc.vector.tensor_copy(och[:], av_ps[:])
                            nc.scalar.dma_start(
                                out=z[6 * h:6 * h + 6], in_=och[:])
                        for nh in range(4):
                            ystg = ap.tile([48, 4, W], BF16, tag="ystg",
                                           bufs=4)
                            for nw in range(4):
                                pj = pap.tile([48, W], F32, tag="pj", bufs=2)
                                nc.tensor.matmul(pj[:], wp[:],
                                                 z[0:48, nh, nw, :],
                                                 start=True, stop=True)
                                nc.vector.tensor_copy(ystg[:, nw, :], pj[:])
                            nc.sync.dma_start(
                                out=y_out[:, nh, :, 4 * ck:4 * ck + 4, :],
                                in_=ystg[:])

    nc.compile()
    nc.generate_event_semaphores()
    return nc


# ====================== host-side helpers ======================

def host_prepare(x, qkv_w, dw_w, proj_w, cfg: Cfg):
    """Full inputs -> list of per-core input dicts."""
    bf = ml_dtypes.bfloat16
    b, cin, H, Wf = x.shape
    assert cin == CIN and Wf == cfg.W
    n = cfg.n_cores
    per_b = n // b
    assert cfg.HLOC * per_b == H

    w1 = qkv_w[:, :, 0, 0].astype(np.float32)          # [144, 48]
    dw = dw_w[:, 0, :, :].astype(np.float32)           # [144, 3, 3]
    # w3T[ic, dy, dx, oc] = w1[oc, ic] * dw[oc, dy, dx]
    w3T = np.einsum('oi,oyx->iyxo', w1, dw)            # [48, 3, 3, 144]

    w3a = np.zeros((128, 3, 96), np.float32)
    w3av = np.zeros((128, 3, 48), np.float32)
    for dy in range(3):
        w3a[0:48, dy] = w3T[:, dy, 0, 0:96]
        w3a[48:96, dy] = w3T[:, dy, 1, 0:96]
        w3a[96:128, dy] = w3T[0:32, dy, 2, 0:96]
        w3av[0:48, dy] = w3T[:, dy, 0, 96:144]
        w3av[48:96, dy] = w3T[:, dy, 1, 96:144]
        w3av[96:128, dy] = w3T[0:32, dy, 2, 96:144]
    w3b = np.zeros((48, 96), np.float32)
    w3bv = np.zeros((48, 48), np.float32)
    for dy in range(3):
        w3b[16 * dy:16 * dy + 16] = w3T[32:48, dy, 2, 0:96]
        w3bv[16 * dy:16 * dy + 16] = w3T[32:48, dy, 2, 96:144]

    wp = np.ascontiguousarray(proj_w[:, :, 0, 0].astype(np.float32).T)
    idm = np.eye(128, dtype=np.float32)

    xp = np.zeros((b, cin, H + 2, cfg.W + 2), dtype=np.float32)
    xp[:, :, 1:H + 1, 1:cfg.W + 1] = x
    xp = xp.astype(bf)

    const = {"w3a": w3a.astype(bf), "w3av": w3av.astype(bf),
             "w3b": w3b.astype(bf), "w3bv": w3bv.astype(bf),
             "wp": wp.astype(bf), "idm": idm.astype(bf)}
    in_maps = []
    for i in range(n):
        bi, r = i // per_b, i % per_b
        xs = np.ascontiguousarray(
            xp[bi, :, cfg.HLOC * r:cfg.HLOC * r + cfg.HLOC + 2, :])
        m = {"x": xs}
        m.update(const)
        in_maps.append(m)
    return in_maps


def host_assemble(results, cfg: Cfg, b, H):
    n = cfg.n_cores
    per_b = n // b
    y = np.empty((b, CIN, H, cfg.W), dtype=np.float32)
    for i in range(n):
        bi, r = i // per_b, i % per_b
        yb = np.asarray(results[i]["y"], dtype=np.float32)
        yc = yb.transpose(0, 3, 1, 4, 2).reshape(CIN, cfg.HLOC, cfg.W)
        y[bi, :, cfg.HLOC * r:cfg.HLOC * (r + 1), :] = yc
    return y


_NC_CACHE = {}


def kernel(x, qkv_w, dw_w, proj_w, temperature):
    x = np.asarray(x, dtype=np.float32)
    qkv_w = np.asarray(qkv_w, dtype=np.float32)
    dw_w = np.asarray(dw_w, dtype=np.float32)
    proj_w = np.asarray(proj_w, dtype=np.float32)
    temperature = np.asarray(temperature, dtype=np.float32)

    from concourse.bass_utils import run_bass_kernel_spmd

    cfg = Cfg(W=512, HLOC=128, n_cores=8)
    temps = tuple(float(t) for t in temperature[:, 0, 0])
    if temps not in _NC_CACHE:
        _NC_CACHE[temps] = build_nc(list(temps), cfg.n_cores)
    nc = _NC_CACHE[temps]
    in_maps = host_prepare(x, qkv_w, dw_w, proj_w, cfg)
    res = run_bass_kernel_spmd(nc, in_maps, list(range(cfg.n_cores)))
    return host_assemble(res.results, cfg, x.shape[0], x.shape[2])


# revision 25
# speedup vs baseline: 1.1736x; 1.1097x over previous
"""nn_CNNxCNN_Attention Trainium2 Bass kernel (self-contained).

Row-sharded data parallelism over 8 cores (core i: batch i//4, rows
128*(i%4)..+128); the only cross-core exchange is an AllReduce of
per-head Gram matrices + squared norms (L2 normalization folds into
the Gram).

Fused 1x1+3x3dw conv via K-packed bf16 matmuls: x is staged in SBUF
2.67x (three column-shifted copies on partitions 0-47/48-95/96-127 plus
a small row-shifted remainder tile) so each matmul contracts K=128 over
(channel x tap) -- 4 matmuls per 512-wide output row per output half
(qk 96 / v 48) instead of 18 K=48 matmuls.

Conv outputs are unsorted into compact persistent staging (qkb_all /
vstg_all) with (nh, nw, h1, w1) free layout; per-head expansion gathers
run on the gpsimd SWDGE queue; q/k chunk transposes use the DMA
crossbar transpose (dma_start_transpose) split across sync/scalar
issue queues; Gram + squared norms stream per superblock overlapping
the next superblock's conv. Softmax folds the L2 norms, temperature,
and 1/rowsum into the transposed attention matrix; attn@v psum is
regrouped by DMA into a (head,chan)-partition tile so the 1x1 proj
runs with contiguous rhs; y is written HBM-blocked and un-blocked on
the host.
"""

from dataclasses import dataclass

import numpy as np
import ml_dtypes

import concourse.bass as bass
import concourse.bacc as bacc
import concourse.mybir as mybir
from concourse.tile import TileContext

F32 = mybir.dt.float32
BF16 = mybir.dt.bfloat16
AF = mybir.ActivationFunctionType
ALU = mybir.AluOpType

HEADS = 8
CIN = 48
W = 512
W1 = 128          # coarse token columns
HLOC = 128        # rows per core
NSB = 4           # superblocks of 32 rows
NCK = 8           # attn/proj token chunks (4 h1-rows of 128 w1 each)


@dataclass
class Cfg:
    W: int = 512
    HLOC: int = 128
    n_cores: int = 8


def build_nc(temps, n_cores=8, stage=3, parts=15):
    # stage: 1=conv+unsort only, 2=+gram (zeros out), 3=full
    # parts bitmask: 1=gathers 2=transposes 4=gram-mm 8=squares
    nc = bacc.Bacc("TRN2", target_bir_lowering=False, debug=False,
                   num_devices=n_cores)

    x_in = nc.declare_dram_parameter("x", [CIN, HLOC + 2, W + 2], BF16,
                                     isOutput=False)
    w3a_in = nc.declare_dram_parameter("w3a", [128, 3, 96], BF16,
                                       isOutput=False)
    w3av_in = nc.declare_dram_parameter("w3av", [128, 3, 48], BF16,
                                        isOutput=False)
    w3b_in = nc.declare_dram_parameter("w3b", [48, 96], BF16, isOutput=False)
    w3bv_in = nc.declare_dram_parameter("w3bv", [48, 48], BF16,
                                        isOutput=False)
    wp_in = nc.declare_dram_parameter("wp", [48, 48], BF16, isOutput=False)
    id_in = nc.declare_dram_parameter("idm", [128, 128], BF16, isOutput=False)
    # blocked output: [ch, nh, nw, h1loc, w1]; host re-interleaves
    y_out = nc.declare_dram_parameter("y", [CIN, 4, 4, 32, 128], BF16,
                                      isOutput=True)

    cc_in = nc.dram_tensor("cc_in", [96, 784], F32)
    cc_out = nc.dram_tensor("cc_out", [96, 784], F32)
    half = n_cores // 2
    groups = [list(range(0, half)), list(range(half, n_cores))]

    with TileContext(nc) as tc:
        with tc.tile_pool(name="persist", bufs=1) as pp:
            # ---- constants ----
            w3a = pp.tile([128, 3, 96], BF16)
            w3av = pp.tile([128, 3, 48], BF16)
            w3b = pp.tile([48, 96], BF16)
            w3bv = pp.tile([48, 48], BF16)
            wp = pp.tile([48, 48], BF16)
            idm = pp.tile([128, 128], BF16)
            nc.sync.dma_start(out=w3a[:], in_=w3a_in[:])
            nc.sync.dma_start(out=w3av[:], in_=w3av_in[:])
            nc.sync.dma_start(out=w3b[:], in_=w3b_in[:])
            nc.sync.dma_start(out=w3bv[:], in_=w3bv_in[:])
            nc.sync.dma_start(out=wp[:], in_=wp_in[:])
            nc.sync.dma_start(out=idm[:], in_=id_in[:])

            # ---- persistent v staging: halves on partitions 0-47/64-111
            # free layout (nh, nw, h1-in-half, w1) ----
            v_sorted = pp.tile([112, 4, 4, 16, W1], BF16)
            sq_part = pp.tile([96, 16, NSB], F32)   # (q8,k8) x superblock
            gsq = pp.tile([96, 784], F32)           # G(768) + sq(16)

            # ================= conv + gram streaming =================
            with tc.tile_pool(name="conv", bufs=2) as cp, \
                 tc.tile_pool(name="psum_conv", bufs=1, space="PSUM") as pcp:
                xrs, xr2s = {}, {}

                def load_group(g):
                    # xr: parts 0-47 = x (dx tap 0), 48-95 = x shifted one
                    # col (dx tap 1), 96-127 = x[0:32] shifted two (dx tap 2)
                    xr = cp.tile([128, 18, 512], BF16, tag="xr")
                    r0 = 16 * g
                    nc.sync.dma_start(out=xr[0:48, :, :],
                                      in_=x_in[:, r0:r0 + 18, 0:512])
                    nc.sync.dma_start(out=xr[48:96, :, :],
                                      in_=x_in[:, r0:r0 + 18, 1:513])
                    nc.sync.dma_start(out=xr[96:128, :, :],
                                      in_=x_in[0:32, r0:r0 + 18, 2:514])
                    # xr2: remainder taps (dy,2) for channels 32-47,
                    # partition block dy holds rows r0+dy+hh*8 .. +8
                    halves = []
                    for hh in range(2):
                        x2 = cp.tile([48, 8, 512], BF16, tag="xr2")
                        for dy in range(3):
                            rb = r0 + dy + 8 * hh
                            nc.scalar.dma_start(
                                out=x2[16 * dy:16 * dy + 16, :, :],
                                in_=x_in[32:48, rb:rb + 8, 2:514])
                        halves.append(x2)
                    xrs[g] = xr
                    xr2s[g] = halves

                def gram_head(sb, h, qkb):
                    qcb = cp.tile([96, 8, W1], BF16, tag="qcb")
                    kcb = cp.tile([96, 8, W1], BF16, tag="kcb")
                    if parts & 1:
                        nc.scalar.dma_start(
                            out=qcb[:], in_=qkb[6 * h:6 * h + 6])
                        nc.scalar.dma_start(
                            out=kcb[:],
                            in_=qkb[48 + 6 * h:48 + 6 * h + 6])
                    else:
                        nc.vector.memset(qcb[:], 0.125)
                        nc.vector.memset(kcb[:], 0.125)
                    qT = cp.tile([128, 8, 96], BF16, tag="qT")
                    kT = cp.tile([128, 8, 96], BF16, tag="kT")
                    if parts & 2:
                        nc.scalar.dma_start_transpose(
                            qT[:], qcb.rearrange("p a b -> p (a b)"))
                        nc.scalar.dma_start_transpose(
                            kT[:], kcb.rearrange("p a b -> p (a b)"))
                    else:
                        nc.vector.memset(qT[:], 0.125)
                        nc.vector.memset(kT[:], 0.125)
                    gps = pcp.tile([96, 96], F32, tag="gps", bufs=2)
                    if parts & 4:
                        for ck in range(8):
                            nc.tensor.matmul(gps[:], kT[:, ck, :],
                                             qT[:, ck, :], start=(ck == 0),
                                             stop=(ck == 7))
                    else:
                        nc.tensor.matmul(gps[:], kT[:, 0, :], qT[:, 0, :],
                                         start=True, stop=True)
                    if sb == 0:
                        nc.vector.tensor_copy(gsq[:, 96 * h:96 * h + 96],
                                              gps[:])
                    else:
                        nc.vector.tensor_tensor(
                            gsq[:, 96 * h:96 * h + 96],
                            gsq[:, 96 * h:96 * h + 96], gps[:], op=ALU.add)
                    if parts & 8:
                        sc0 = cp.tile([96, 8 * W1], BF16, tag="sc0", bufs=2)
                        nc.gpsimd.tensor_tensor(
                            sc0[:], qcb.rearrange("p a b -> p (a b)"),
                            qcb.rearrange("p a b -> p (a b)"), op=ALU.mult)
                        nc.vector.tensor_reduce(
                            sq_part[:, h, sb:sb + 1], sc0[:],
                            axis=mybir.AxisListType.X, op=ALU.add)
                        sc1 = cp.tile([96, 8 * W1], BF16, tag="sc1", bufs=2)
                        nc.scalar.activation(
                            sc1[:], kcb.rearrange("p a b -> p (a b)"),
                            AF.Square,
                            accum_out=sq_part[:, 8 + h, sb:sb + 1])
                    else:
                        nc.vector.memset(sq_part[:, h, sb:sb + 1], 1.0)
                        nc.vector.memset(sq_part[:, 8 + h, sb:sb + 1], 1.0)

                load_group(0)
                qkbs = {}
                for sb in range(NSB):
                    # superblock qk staging, (nh, nw, gg, grprow, w1)
                    qkbs[sb] = cp.tile([96, 4, 4, 2, 4, W1], BF16, tag="qkb",
                                       name=f"qkb{sb}")
                    qkb = qkbs[sb]
                    for gg in range(2):
                        g = 2 * sb + gg
                        xr, xr2 = xrs.pop(g), xr2s.pop(g)
                        for yy in range(16):
                            if yy == 8 and g + 1 < 2 * NSB:
                                load_group(g + 1)
                            # software-pipelined gram of the previous
                            # superblock: one head per 4 conv rows
                            r32 = 16 * gg + yy
                            if (stage >= 2 and sb >= 1 and r32 % 4 == 2):
                                gram_head(sb - 1, r32 // 4, qkbs[sb - 1])
                            x2 = xr2[yy // 8]
                            yl = yy % 8
                            y = 16 * g + yy
                            vb = 0 if y < 64 else 64
                            ps_qk = pcp.tile([96, W], F32, tag="ps_qk",
                                             bufs=3)
                            ps_v = pcp.tile([112, W], F32, tag="ps_v",
                                            bufs=3)
                            for dy in range(3):
                                rhs = xr[0:128, yy + dy, 0:512]
                                nc.tensor.matmul(ps_qk[:], w3a[:, dy, :], rhs,
                                                 start=(dy == 0), stop=False)
                            nc.tensor.matmul(ps_qk[:], w3b[:],
                                             x2[0:48, yl, :],
                                             start=False, stop=True)
                            vs = ps_v[vb:vb + 48, :]
                            for dy in range(3):
                                rhs = xr[0:128, yy + dy, 0:512]
                                nc.tensor.matmul(vs, w3av[:, dy, :], rhs,
                                                 start=(dy == 0), stop=False,
                                                 tile_position=(0, vb))
                            nc.tensor.matmul(vs, w3bv[:],
                                             x2[0:48, yl, :],
                                             start=False, stop=True,
                                             tile_position=(0, vb))
                            nh, grprow = yy % 4, yy // 4
                            src_qk = ps_qk.rearrange("p (w1 nw) -> p nw w1",
                                                     nw=4)
                            nc.vector.tensor_copy(
                                qkb[:, nh, :, gg, grprow, :], src_qk)
                            src_v = vs.rearrange("p (w1 nw) -> p nw w1",
                                                 nw=4)
                            nc.vector.tensor_copy(
                                v_sorted[vb:vb + 48, nh, :,
                                         (y % 64) // 4, :], src_v)

                # last superblock's gram (exposed, no conv cover left)
                for h in range(HEADS if stage >= 2 else 0):
                    gram_head(NSB - 1, h, qkbs[NSB - 1])

            # ================= collective + tail =================
            if stage < 3:
                with tc.tile_pool(name="zf", bufs=1) as zp:
                    zt = zp.tile([CIN, 4, 4, W], BF16)
                    nc.vector.memset(zt[:], 0.0)
                    for ck in range(NCK):
                        nc.sync.dma_start(
                            out=y_out[:, :, :, 4 * ck:4 * ck + 4, :],
                            in_=zt[:])
            if stage >= 3:
              with tc.tile_pool(name="tailp", bufs=1) as tp:
                nc.vector.tensor_reduce(gsq[:, 768:784], sq_part[:],
                                        axis=mybir.AxisListType.X, op=ALU.add)
                nc.sync.dma_start(out=cc_in[:], in_=gsq[:])
                nc.gpsimd.collective_compute(
                    "AllReduce", ALU.add, replica_groups=groups,
                    ins=[cc_in[:]], outs=[cc_out[:]])
                gsq_r = tp.tile([96, 784], F32)
                nc.sync.dma_start(out=gsq_r[:], in_=cc_out[:])

                # vcb expansion gathers (hide under the collective)
                vcbs = {}
                for h in range(HEADS):
                    vcbs[h] = tp.tile([96, 32, W1], BF16, name=f"vcb{h}")
                    nc.sync.dma_start(out=vcbs[h][:, 0:16, :],
                                      in_=v_sorted[6 * h:6 * h + 6])
                    nc.sync.dma_start(
                        out=vcbs[h][:, 16:32, :],
                        in_=v_sorted[64 + 6 * h:64 + 6 * h + 6])

                # ============ softmax (norm + 1/rowsum folded) ============
                nrm = tp.tile([96, 16], F32)
                rs = tp.tile([96, 16], F32)
                nc.scalar.sqrt(nrm[:], gsq_r[:, 768:784])
                nc.vector.tensor_scalar_max(nrm[:], nrm[:], 1e-12)
                nc.vector.reciprocal(rs[:], nrm[:])

                atTs = {}
                with tc.tile_pool(name="smx", bufs=2) as sp, \
                     tc.tile_pool(name="psum_smx", bufs=2,
                                  space="PSUM") as psp:
                    for h in range(HEADS):
                        # gsq holds G^T[e,d]; scale rows (e) by k-norms
                        hbf = sp.tile([96, 96], BF16, tag="hbf")
                        nc.vector.tensor_scalar_mul(
                            hbf[:], gsq_r[:, 96 * h:96 * h + 96],
                            rs[:, 8 + h:8 + h + 1])
                        ht_ps = psp.tile([96, 96], BF16, tag="ht_ps")
                        nc.tensor.transpose(ht_ps[:], hbf[:], idm[0:96, 0:96])
                        sd = sp.tile([96, 1], F32, tag="sd")
                        nc.vector.tensor_scalar_mul(sd[:], rs[:, h:h + 1],
                                                    float(temps[h]))
                        aexp = sp.tile([96, 96], BF16, tag="aexp")
                        rowsum = sp.tile([96, 1], F32, tag="rowsum")
                        nc.scalar.activation(aexp[:], ht_ps[:], AF.Exp,
                                             scale=sd[:], accum_out=rowsum[:])
                        rinv = sp.tile([96, 1], F32, tag="rinv")
                        nc.vector.reciprocal(rinv[:], rowsum[:])
                        aexp2 = sp.tile([96, 96], BF16, tag="aexp2")
                        nc.vector.tensor_scalar_mul(aexp2[:], aexp[:],
                                                    rinv[:])
                        at_ps = psp.tile([96, 96], BF16, tag="at_ps")
                        nc.tensor.transpose(at_ps[:], aexp2[:],
                                            idm[0:96, 0:96])
                        atT = tp.tile([96, 96], BF16, name=f"atT{h}")
                        nc.scalar.copy(atT[:], at_ps[:])
                        atTs[h] = atT

                # ============ attn@v + regroup + proj ============
                with tc.tile_pool(name="attn", bufs=2) as ap, \
                     tc.tile_pool(name="psum_attn", bufs=2,
                                  space="PSUM") as pap:
                    def attn_ck(ck, z):
                        for h in range(HEADS):
                            av_ps = pap.tile([96, W], F32, tag="av_ps",
                                             bufs=2)
                            rhs = vcbs[h][:, 4 * ck:4 * ck + 4, :].rearrange(
                                "p a b -> p (a b)")
                            nc.tensor.matmul(av_ps[:], atTs[h][:], rhs,
                                             start=True, stop=True)
                            och = ap.tile([96, W], BF16, tag="och", bufs=4)
                            nc.vector.tensor_copy(och[:], av_ps[:])
                            nc.scalar.dma_start(
                                out=z[6 * h:6 * h + 6], in_=och[:])

                    def proj_ck(ck, z):
                        for nh in range(4):
                            ystg = ap.tile([48, 4, W], BF16, tag="ystg",
                                           bufs=4)
                            for nw in range(4):
                                pj = pap.tile([48, W], F32, tag="pj", bufs=2)
                                nc.tensor.matmul(pj[:], wp[:],
                                                 z[0:48, nh, nw, :],
                                                 start=True, stop=True)
                                nc.vector.tensor_copy(ystg[:, nw, :], pj[:])
                            nc.sync.dma_start(
                                out=y_out[:, nh, :, 4 * ck:4 * ck + 4, :],
                                in_=ystg[:])

                    zs = {}
                    for ck in range(NCK):
                        zs[ck] = ap.tile([48, 4, 4, W], BF16, tag="z",
                                         name=f"z{ck}")
                        attn_ck(ck, zs[ck])
                        if ck >= 1:
                            proj_ck(ck - 1, zs.pop(ck - 1))
                    proj_ck(NCK - 1, zs.pop(NCK - 1))

    nc.compile()
    nc.generate_event_semaphores()
    return nc


# ====================== host-side helpers ======================

def host_prepare(x, qkv_w, dw_w, proj_w, cfg: Cfg):
    """Full inputs -> list of per-core input dicts."""
    bf = ml_dtypes.bfloat16
    b, cin, H, Wf = x.shape
    assert cin == CIN and Wf == cfg.W
    n = cfg.n_cores
    per_b = n // b
    assert cfg.HLOC * per_b == H

    w1 = qkv_w[:, :, 0, 0].astype(np.float32)          # [144, 48]
    dw = dw_w[:, 0, :, :].astype(np.float32)           # [144, 3, 3]
    # w3T[ic, dy, dx, oc] = w1[oc, ic] * dw[oc, dy, dx]
    w3T = np.einsum('oi,oyx->iyxo', w1, dw)            # [48, 3, 3, 144]

    w3a = np.zeros((128, 3, 96), np.float32)
    w3av = np.zeros((128, 3, 48), np.float32)
    for dy in range(3):
        w3a[0:48, dy] = w3T[:, dy, 0, 0:96]
        w3a[48:96, dy] = w3T[:, dy, 1, 0:96]
        w3a[96:128, dy] = w3T[0:32, dy, 2, 0:96]
        w3av[0:48, dy] = w3T[:, dy, 0, 96:144]
        w3av[48:96, dy] = w3T[:, dy, 1, 96:144]
        w3av[96:128, dy] = w3T[0:32, dy, 2, 96:144]
    w3b = np.zeros((48, 96), np.float32)
    w3bv = np.zeros((48, 48), np.float32)
    for dy in range(3):
        w3b[16 * dy:16 * dy + 16] = w3T[32:48, dy, 2, 0:96]
        w3bv[16 * dy:16 * dy + 16] = w3T[32:48, dy, 2, 96:144]

    wp = np.ascontiguousarray(proj_w[:, :, 0, 0].astype(np.float32).T)
    idm = np.eye(128, dtype=np.float32)

    xp = np.zeros((b, cin, H + 2, cfg.W + 2), dtype=np.float32)
    xp[:, :, 1:H + 1, 1:cfg.W + 1] = x
    xp = xp.astype(bf)

    const = {"w3a": w3a.astype(bf), "w3av": w3av.astype(bf),
             "w3b": w3b.astype(bf), "w3bv": w3bv.astype(bf),
             "wp": wp.astype(bf), "idm": idm.astype(bf)}
    in_maps = []
    for i in range(n):
        bi, r = i // per_b, i % per_b
        xs = np.ascontiguousarray(
            xp[bi, :, cfg.HLOC * r:cfg.HLOC * r + cfg.HLOC + 2, :])
        m = {"x": xs}
        m.update(const)
        in_maps.append(m)
    return in_maps


def host_assemble(results, cfg: Cfg, b, H):
    n = cfg.n_cores
    per_b = n // b
    y = np.empty((b, CIN, H, cfg.W), dtype=np.float32)
    for i in range(n):
        bi, r = i // per_b, i % per_b
        yb = np.asarray(results[i]["y"], dtype=np.float32)
        yc = yb.transpose(0, 3, 1, 4, 2).reshape(CIN, cfg.HLOC, cfg.W)
        y[bi, :, cfg.HLOC * r:cfg.HLOC * (r + 1), :] = yc
    return y


_NC_CACHE = {}


def kernel(x, qkv_w, dw_w, proj_w, temperature):
    x = np.asarray(x, dtype=np.float32)
    qkv_w = np.asarray(qkv_w, dtype=np.float32)
    dw_w = np.asarray(dw_w, dtype=np.float32)
    proj_w = np.asarray(proj_w, dtype=np.float32)
    temperature = np.asarray(temperature, dtype=np.float32)

    from concourse.bass_utils import run_bass_kernel_spmd

    cfg = Cfg(W=512, HLOC=128, n_cores=8)
    temps = tuple(float(t) for t in temperature[:, 0, 0])
    if temps not in _NC_CACHE:
        _NC_CACHE[temps] = build_nc(list(temps), cfg.n_cores)
    nc = _NC_CACHE[temps]
    in_maps = host_prepare(x, qkv_w, dw_w, proj_w, cfg)
    res = run_bass_kernel_spmd(nc, in_maps, list(range(cfg.n_cores)))
    return host_assemble(res.results, cfg, x.shape[0], x.shape[2])
